# revision 6
# baseline (speedup 1.0000x reference)
"""NCNPredictor Trainium2 kernel: bit-packed adjacency + DVE extract/MAC.

out[e] = xij(e) + sum_n [ yA(n)*(b0-b0b1) + yB(n)*b1 + yC(n)*(b2-b0b2) ] + b
with b0 = a01[i,n]*a01[j,n], b1 = a1[..], b2 = a012[..]. The correction
products factor per side (b0b1 = (a01*a1)[i]*(a01*a1)[j]), giving 5 bilinear
channels, each an AND of per-side bits:
   bits5 = p | q<<1 | r<<2 | (p&q)<<3 | (p&r)<<4
Two adjacency columns are packed per int16 lane (even col -> bits 0-4, odd
col -> bits 8-12). Per 128-edge tile the kernel gathers rows i and j of the
packed table (indirect DMA), computes w = gi & gj (one tensor_tensor), then
for each of the 10 (channel, parity) pairs: a tensor_scalar AND extracts the
mask plane (4x DVE mode) and a scalar_tensor_tensor computes
(mask * sgn/2^bit) * y with a fused row-sum accumulator (2x mode). A final
tensor_reduce adds the 10 channel sums. y vectors (x @ Wxs blocks), the xij
dot products, and the bias are precomputed on the host, mirroring the
reference's host-side weight algebra.

Sharding: target edges split across the 8 cores (1024 each); each core scans
all N adjacency columns of its own edges, so no cross-core reduction.
"""

import sys
from contextlib import ExitStack

import numpy as np

sys.path.insert(0, "/opt/trn_rl_repo")

import concourse.bass as bass
import concourse.tile as tile
from concourse import bacc, mybir
from concourse.bass_utils import run_bass_kernel_spmd

N = 10000
D = 128
E = 8192
NCORES = 8
E_OWN = E // NCORES          # 1024 edges per core
P = 128
T = E_OWN // P               # 8 tiles per core
WL = N // 2                  # 5000 int16 lanes of packed adjacency
NCH = 10                     # 5 channels x 2 column parities
F32 = mybir.dt.float32
BF16 = mybir.dt.bfloat16
I16 = mybir.dt.int16
I32 = mybir.dt.int32
MUL = mybir.AluOpType.mult
ADD = mybir.AluOpType.add
AND = mybir.AluOpType.bitwise_and

# (Yvec 0=A,1=B,2=C, sign, bit position); k<5 -> even columns, k>=5 -> odd
CHANNELS = [
    (0, 1.0, 0), (1, 1.0, 1), (2, 1.0, 2), (0, -1.0, 3), (2, -1.0, 4),
    (0, 1.0, 8), (1, 1.0, 9), (2, 1.0, 10), (0, -1.0, 11), (2, -1.0, 12),
]
BITVALS = [1 << c[2] for c in CHANNELS]

_CACHE = {}


def _build_nc(reps=1):
    nc = bacc.Bacc(num_swdge_queues=4)

    tableA = nc.declare_dram_parameter("tableA", [N, WL], I16, False)
    ycat = nc.declare_dram_parameter("ycat", [P, 6 * WL], BF16, False)
    bitc = nc.declare_dram_parameter("bitc", [P, NCH], I16, False)
    iall = nc.declare_dram_parameter("iall", [P, T], I32, False)
    jall = nc.declare_dram_parameter("jall", [P, T], I32, False)
    outb = nc.declare_dram_parameter("outb", [P, T], F32, True)

    with tile.TileContext(nc) as tc, ExitStack() as ctx:
        const = ctx.enter_context(tc.tile_pool(name="const", bufs=1))
        yk = []
        for u in range(6):
            y_t = const.tile([P, WL], BF16, name=f"y{u}")
            nc.sync.dma_start(y_t[:], ycat[:, u * WL : (u + 1) * WL])
            yk.append(y_t)
        bitc_t = const.tile([P, NCH], I16, name="bitc_t")
        nc.sync.dma_start(bitc_t[:], bitc[:])
        iall_t = const.tile([P, T], I32, name="iall_t")
        nc.sync.dma_start(iall_t[:], iall[:])
        jall_t = const.tile([P, T], I32, name="jall_t")
        nc.sync.dma_start(jall_t[:], jall[:])

        gip = ctx.enter_context(tc.tile_pool(name="gip", bufs=2))
        gjp = ctx.enter_context(tc.tile_pool(name="gjp", bufs=2))
        wp = ctx.enter_context(tc.tile_pool(name="wp", bufs=2))
        scrd = ctx.enter_context(tc.tile_pool(name="scrd", bufs=2))
        actp = ctx.enter_context(tc.tile_pool(name="actp", bufs=2))
        accp = ctx.enter_context(tc.tile_pool(name="accp", bufs=2))
        outp = ctx.enter_context(tc.tile_pool(name="outp", bufs=1))

        outb_t = outp.tile([P, T], F32, name="outb_t")

        for t in range(T * reps):
            t = t % T
            ioff = bass.IndirectOffsetOnAxis(ap=iall_t[:, t : t + 1], axis=0)
            joff = bass.IndirectOffsetOnAxis(ap=jall_t[:, t : t + 1], axis=0)

            gi = gip.tile([P, WL], I16, name="gi")
            nc.gpsimd.indirect_dma_start(
                out=gi[:], out_offset=None, in_=tableA[:], in_offset=ioff)
            gj = gjp.tile([P, WL], I16, name="gj")
            nc.gpsimd.indirect_dma_start(
                out=gj[:], out_offset=None, in_=tableA[:], in_offset=joff)
            w = wp.tile([P, WL], I16, name="w")
            nc.vector.tensor_tensor(out=w[:], in0=gi[:], in1=gj[:], op=AND)

            acc = accp.tile([P, NCH], F32, name="acc")
            for k in range(NCH):
                mi, sgn, bit = CHANNELS[k]
                par = 0 if k < 5 else 1
                m = scrd.tile([P, WL], I16, name="m")
                nc.vector.tensor_scalar(
                    out=m[:], in0=w[:], scalar1=bitc_t[:, k : k + 1],
                    scalar2=None, op0=AND,
                )
                # product on DVE at 2x (mask {0,2^bit} * y, exact in bf16);
                # the row-sum runs on the ACT engine in parallel, with the
                # sign/2^bit scale folded into its scale slot.
                s = scrd.tile([P, WL], BF16, name="s")
                nc.vector.tensor_tensor(
                    out=s[:], in0=m[:], in1=yk[par * 3 + mi][:], op=MUL,
                )
                ao = actp.tile([P, WL], BF16, name="ao")
                nc.scalar.activation(
                    out=ao[:], in_=s[:],
                    func=mybir.ActivationFunctionType.Copy,
                    bias=0.0, scale=float(sgn / (1 << bit)),
                    accum_out=acc[:, k : k + 1],
                )
            nc.vector.tensor_reduce(
                out=outb_t[:, t : t + 1], in_=acc[:], axis=mybir.AxisListType.X,
                op=ADD,
            )

        nc.sync.dma_start(outb[:], outb_t[:])

    return nc


def get_nc(reps=1):
    key = f"nc{reps}"
    if key not in _CACHE:
        nc = _build_nc(reps)
        nc.compile()
        _CACHE[key] = nc
    return _CACHE[key]


def make_in_maps(x, adj_0_1, adj_1, adj_0_1_2, tar_ei, Wxs, bxs):
    import ml_dtypes

    bf = ml_dtypes.bfloat16
    x32 = np.ascontiguousarray(x, dtype=np.float32)
    wxs = np.asarray(Wxs, dtype=np.float32)
    w0 = wxs[0:D, 0]
    Y = x32 @ np.concatenate(
        [wxs[D : 2 * D], wxs[2 * D : 3 * D], wxs[3 * D : 4 * D]], axis=1
    )  # [N, 3] f32

    p = (np.asarray(adj_0_1) != 0)
    q = (np.asarray(adj_1) != 0)
    r = (np.asarray(adj_0_1_2) != 0)
    bits = (
        p.astype(np.uint8)
        | (q.astype(np.uint8) << 1)
        | (r.astype(np.uint8) << 2)
        | ((p & q).astype(np.uint8) << 3)
        | ((p & r).astype(np.uint8) << 4)
    )
    tableA = (
        bits[:, 0::2].astype(np.uint16) | (bits[:, 1::2].astype(np.uint16) << 8)
    ).view(np.int16)  # [N, WL]

    ycat = np.empty((P, 6 * WL), dtype=bf)
    for par in range(2):
        for mi in range(3):
            u = par * 3 + mi
            ycat[:, u * WL : (u + 1) * WL] = Y[par::2, mi].astype(bf)[None, :]

    bitc = np.broadcast_to(
        np.asarray(BITVALS, dtype=np.int16)[None, :], (P, NCH)
    ).copy()

    ii = np.asarray(tar_ei[0], dtype=np.int32)
    jj = np.asarray(tar_ei[1], dtype=np.int32)

    # xij term on the host (tiny E x D gather-dot), added in combine_results
    xw = x32 * w0[None, :]
    xij = np.einsum(
        "ed,ed->e", xw[ii].astype(np.float32), x32[jj].astype(np.float32)
    ).astype(np.float64)

    # Per-core edge sort by source row: i-side gathers walk the table in
    # ascending row order (better HBM page locality). Un-permuted in
    # combine_results.
    in_maps = []
    perms = []
    for c in range(NCORES):
        esl = slice(c * E_OWN, (c + 1) * E_OWN)
        ic, jc = ii[esl], jj[esl]
        order = np.argsort(ic, kind="stable")
        perms.append(order)
        in_maps.append({
            "tableA": tableA,
            "ycat": ycat,
            "bitc": bitc,
            "iall": np.ascontiguousarray(ic[order].reshape(T, P).T),
            "jall": np.ascontiguousarray(jc[order].reshape(T, P).T),
        })
    _CACHE["xij"] = xij
    _CACHE["perms"] = perms
    return in_maps


def combine_results(results, b):
    parts = []
    for c in range(NCORES):
        r = np.asarray(results[c]["outb"], dtype=np.float64)  # [P, T]
        vals = r.T.reshape(E_OWN)  # in sorted-edge order
        unperm = np.empty(E_OWN, dtype=np.float64)
        unperm[_CACHE["perms"][c]] = vals
        parts.append(unperm)
    out = np.concatenate(parts) + _CACHE["xij"] + b
    return out.astype(np.float32).reshape(E, 1)


def kernel(x, adj_0_1, adj_1, adj_0_1_2, tar_ei, Wxs, bxs):
    nc = get_nc()
    in_maps = make_in_maps(x, adj_0_1, adj_1, adj_0_1_2, tar_ei, Wxs, bxs)
    res = run_bass_kernel_spmd(nc, in_maps, list(range(NCORES)))
    b = float(np.asarray(bxs, dtype=np.float32).reshape(-1)[0])
    return combine_results(res.results, b)


# revision 8
# speedup vs baseline: 2.1456x; 2.1456x over previous
"""NCNPredictor Trainium2 kernel: bit-packed adjacency + DVE extract/MAC.

out[e] = xij(e) + sum_n [ yA(n)*(b0-b0b1) + yB(n)*b1 + yC(n)*(b2-b0b2) ] + b
with b0 = a01[i,n]*a01[j,n], b1 = a1[..], b2 = a012[..]. The correction
products factor per side (b0b1 = (a01*a1)[i]*(a01*a1)[j]), giving 5 bilinear
channels, each an AND of per-side bits:
   bits5 = p | q<<1 | r<<2 | (p&q)<<3 | (p&r)<<4
Two adjacency columns are packed per int16 lane (even col -> bits 0-4, odd
col -> bits 8-12). Per 128-edge tile the kernel gathers rows i and j of the
packed table (indirect DMA), computes w = gi & gj (one tensor_tensor), then
for each of the 10 (channel, parity) pairs: a tensor_scalar AND extracts the
mask plane (4x DVE mode) and a scalar_tensor_tensor computes
(mask * sgn/2^bit) * y with a fused row-sum accumulator (2x mode). A final
tensor_reduce adds the 10 channel sums. y vectors (x @ Wxs blocks), the xij
dot products, and the bias are precomputed on the host, mirroring the
reference's host-side weight algebra.

Sharding: target edges split across the 8 cores (1024 each); each core scans
all N adjacency columns of its own edges, so no cross-core reduction.
"""

import sys
from contextlib import ExitStack

import numpy as np

sys.path.insert(0, "/opt/trn_rl_repo")

import concourse.bass as bass
import concourse.tile as tile
from concourse import bacc, mybir
from concourse.bass_utils import run_bass_kernel_spmd

N = 10000
D = 128
E = 8192
NCORES = 8
E_OWN = E // NCORES          # 1024 edges per core
P = 128
T = E_OWN // P               # 8 tiles per core
WL = N // 2                  # 5000 int16 lanes of packed adjacency
NCH = 10                     # 5 channels x 2 column parities
F32 = mybir.dt.float32
BF16 = mybir.dt.bfloat16
I16 = mybir.dt.int16
I32 = mybir.dt.int32
MUL = mybir.AluOpType.mult
ADD = mybir.AluOpType.add
AND = mybir.AluOpType.bitwise_and

# (Yvec 0=A,1=B,2=C, sign, bit position); k<5 -> even columns, k>=5 -> odd
CHANNELS = [
    (0, 1.0, 0), (1, 1.0, 1), (2, 1.0, 2), (0, -1.0, 3), (2, -1.0, 4),
    (0, 1.0, 8), (1, 1.0, 9), (2, 1.0, 10), (0, -1.0, 11), (2, -1.0, 12),
]
BITVALS = [1 << c[2] for c in CHANNELS]

_CACHE = {}


def _build_nc(reps=1):
    nc = bacc.Bacc(num_swdge_queues=4)

    tableA = nc.declare_dram_parameter("tableA", [N, WL], I16, False)
    ycat = nc.declare_dram_parameter("ycat", [P, 6 * WL], BF16, False)
    bitc = nc.declare_dram_parameter("bitc", [P, NCH], I16, False)
    iall = nc.declare_dram_parameter("iall", [P, T], I32, False)
    jall = nc.declare_dram_parameter("jall", [P, T], I32, False)
    outb = nc.declare_dram_parameter("outb", [P, T], F32, True)

    with tile.TileContext(nc) as tc, ExitStack() as ctx:
        const = ctx.enter_context(tc.tile_pool(name="const", bufs=1))
        yk = []
        for u in range(6):
            y_t = const.tile([P, WL], BF16, name=f"y{u}")
            nc.sync.dma_start(y_t[:], ycat[:, u * WL : (u + 1) * WL])
            yk.append(y_t)
        bitc_t = const.tile([P, NCH], I16, name="bitc_t")
        nc.sync.dma_start(bitc_t[:], bitc[:])
        iall_t = const.tile([P, T], I32, name="iall_t")
        nc.sync.dma_start(iall_t[:], iall[:])
        jall_t = const.tile([P, T], I32, name="jall_t")
        nc.sync.dma_start(jall_t[:], jall[:])

        gip = ctx.enter_context(tc.tile_pool(name="gip", bufs=2))
        gjp = ctx.enter_context(tc.tile_pool(name="gjp", bufs=2))
        wp = ctx.enter_context(tc.tile_pool(name="wp", bufs=2))
        scrd = ctx.enter_context(tc.tile_pool(name="scrd", bufs=2))
        actp = ctx.enter_context(tc.tile_pool(name="actp", bufs=2))
        accp = ctx.enter_context(tc.tile_pool(name="accp", bufs=2))
        outp = ctx.enter_context(tc.tile_pool(name="outp", bufs=1))

        outb_t = outp.tile([P, T], F32, name="outb_t")

        for t in range(T * reps):
            t = t % T
            ioff = bass.IndirectOffsetOnAxis(ap=iall_t[:, t : t + 1], axis=0)
            joff = bass.IndirectOffsetOnAxis(ap=jall_t[:, t : t + 1], axis=0)

            gi = gip.tile([P, WL], I16, name="gi")
            nc.gpsimd.indirect_dma_start(
                out=gi[:], out_offset=None, in_=tableA[:], in_offset=ioff)
            gj = gjp.tile([P, WL], I16, name="gj")
            nc.gpsimd.indirect_dma_start(
                out=gj[:], out_offset=None, in_=tableA[:], in_offset=joff)
            w = wp.tile([P, WL], I16, name="w")
            nc.vector.tensor_tensor(out=w[:], in0=gi[:], in1=gj[:], op=AND)

            acc = accp.tile([P, NCH], F32, name="acc")
            for k in range(NCH):
                mi, sgn, bit = CHANNELS[k]
                par = 0 if k < 5 else 1
                m = scrd.tile([P, WL], I16, name="m")
                nc.vector.tensor_scalar(
                    out=m[:], in0=w[:], scalar1=bitc_t[:, k : k + 1],
                    scalar2=None, op0=AND,
                )
                # product on DVE at 2x (mask {0,2^bit} * y, exact in bf16);
                # the row-sum runs on the ACT engine in parallel, with the
                # sign/2^bit scale folded into its scale slot.
                s = scrd.tile([P, WL], BF16, name="s")
                nc.vector.tensor_tensor(
                    out=s[:], in0=m[:], in1=yk[par * 3 + mi][:], op=MUL,
                )
                ao = actp.tile([P, WL], BF16, name="ao")
                nc.scalar.activation(
                    out=ao[:], in_=s[:],
                    func=mybir.ActivationFunctionType.Copy,
                    bias=0.0, scale=float(sgn / (1 << bit)),
                    accum_out=acc[:, k : k + 1],
                )
            nc.vector.tensor_reduce(
                out=outb_t[:, t : t + 1], in_=acc[:], axis=mybir.AxisListType.X,
                op=ADD,
            )

        nc.sync.dma_start(outb[:], outb_t[:])

    return nc


def get_nc(reps=1):
    key = f"nc{reps}"
    if key not in _CACHE:
        nc = _build_nc(reps)
        nc.compile()
        _CACHE[key] = nc
    return _CACHE[key]


def make_in_maps(x, adj_0_1, adj_1, adj_0_1_2, tar_ei, Wxs, bxs):
    import ml_dtypes

    bf = ml_dtypes.bfloat16
    x32 = np.ascontiguousarray(x, dtype=np.float32)
    wxs = np.asarray(Wxs, dtype=np.float32)
    w0 = wxs[0:D, 0]
    Y = x32 @ np.concatenate(
        [wxs[D : 2 * D], wxs[2 * D : 3 * D], wxs[3 * D : 4 * D]], axis=1
    )  # [N, 3] f32

    p = (np.asarray(adj_0_1) != 0)
    q = (np.asarray(adj_1) != 0)
    r = (np.asarray(adj_0_1_2) != 0)
    bits = (
        p.astype(np.uint8)
        | (q.astype(np.uint8) << 1)
        | (r.astype(np.uint8) << 2)
        | ((p & q).astype(np.uint8) << 3)
        | ((p & r).astype(np.uint8) << 4)
    )
    tableA = (
        bits[:, 0::2].astype(np.uint16) | (bits[:, 1::2].astype(np.uint16) << 8)
    ).view(np.int16)  # [N, WL]

    ycat = np.empty((P, 6 * WL), dtype=bf)
    for par in range(2):
        for mi in range(3):
            u = par * 3 + mi
            ycat[:, u * WL : (u + 1) * WL] = Y[par::2, mi].astype(bf)[None, :]

    bitc = np.broadcast_to(
        np.asarray(BITVALS, dtype=np.int16)[None, :], (P, NCH)
    ).copy()

    ii = np.asarray(tar_ei[0], dtype=np.int32)
    jj = np.asarray(tar_ei[1], dtype=np.int32)

    # xij term on the host (tiny E x D gather-dot), added in combine_results
    xw = x32 * w0[None, :]
    xij = np.einsum(
        "ed,ed->e", xw[ii].astype(np.float32), x32[jj].astype(np.float32)
    ).astype(np.float64)

    # Per-core edge sort by source row: i-side gathers walk the table in
    # ascending row order (better HBM page locality). Un-permuted in
    # combine_results.
    in_maps = []
    perms = []
    for c in range(NCORES):
        esl = slice(c * E_OWN, (c + 1) * E_OWN)
        ic, jc = ii[esl], jj[esl]
        order = np.argsort(ic, kind="stable")
        perms.append(order)
        in_maps.append({
            "tableA": tableA,
            "ycat": ycat,
            "bitc": bitc,
            "iall": np.ascontiguousarray(ic[order].reshape(T, P).T),
            "jall": np.ascontiguousarray(jc[order].reshape(T, P).T),
        })
    _CACHE["xij"] = xij
    _CACHE["perms"] = perms
    return in_maps


def combine_results(results, b):
    parts = []
    for c in range(NCORES):
        r = np.asarray(results[c]["outb"], dtype=np.float64)  # [P, T]
        vals = r.T.reshape(E_OWN)  # in sorted-edge order
        unperm = np.empty(E_OWN, dtype=np.float64)
        unperm[_CACHE["perms"][c]] = vals
        parts.append(unperm)
    out = np.concatenate(parts) + _CACHE["xij"] + b
    return out.astype(np.float32).reshape(E, 1)


def kernel(x, adj_0_1, adj_1, adj_0_1_2, tar_ei, Wxs, bxs):
    nc = get_nc()
    in_maps = make_in_maps(x, adj_0_1, adj_1, adj_0_1_2, tar_ei, Wxs, bxs)
    res = run_bass_kernel_spmd(nc, in_maps, list(range(NCORES)))
    b = float(np.asarray(bxs, dtype=np.float32).reshape(-1)[0])
    return combine_results(res.results, b)
